# revision 23
# baseline (speedup 1.0000x reference)
"""TRN2 Bass kernel for multi-head self-attention with RoPE (causal).

Problem: B=4, S=2048, D=768, H=12 heads of dk=64, fp32 in/out.

Sharding: 8 cores = 4 batches x 2 head-groups of 6 heads. Each core computes
QKV projections for its 6 heads, RoPE, causal flash-ish attention, and a
partial output projection; the host sums the two partials per batch.

Numerics: split-bf16 (hi+lo) 3-term matmuls for the Q/K projections and
for Q.K^T scores (the softmax here is argmax-like: score std ~600, so
bf16/tf32 rounding would flip winners). V/AV/O-proj in plain bf16.

Layouts (per core, device side):
  xh/xl      [128, 6, S]   bf16  x^T tiled on d_in (contraction on partitions)
  wqh/...    [128, 6, 384] bf16  W^T tiled on d_in; q/k rows rope-permuted
  wvT        [128, 6, 384] bf16
  woT        [128, 3, 768] bf16  (c = head-concat dim tiled)
  qT/kT      computed as [dh=2x64 heads ("pair"), s] with d on partitions
  scores     psum [128 q, 2 heads, kc] fp32, softmax over free dim
  p^T        via PE transpose of exp(scores) -> AV matmuls produce
  avT        psum [65, 2, 128]: rows 0..63 = sum_k p*v, row 64 = sum_k p
"""

import sys

sys.path.insert(0, "/opt/trn_rl_repo")

from contextlib import ExitStack

import ml_dtypes
import numpy as np

import concourse.bass as bass
import concourse.tile as tile
from concourse import bacc, mybir
from concourse.bass_utils import run_bass_kernel_spmd

F32 = mybir.dt.float32
BF16 = mybir.dt.bfloat16
bf16 = ml_dtypes.bfloat16

B, D, H, DK = 4, 768, 12, 64
NHC = 6          # heads per core
NPAIR = 3        # head pairs per core
DSUB = 6         # d_in subtiles of 128
CPC = NHC * DK   # 384 head-dims per core


def _build(S=2048, CHUNK=1024, trace_label="", debug_stop=""):
    """Build the SPMD program (pair-pipelined: proj of pair p+1 overlaps
    attention of pair p, keeping the PE dense so HAM stays at full clock)."""
    NQT = S // 128
    nc = bacc.Bacc("TRN2", target_bir_lowering=False, debug=False, num_devices=8)

    def din(name, shape, dt):
        return nc.dram_tensor(name, shape, dt, kind="ExternalInput").ap()

    xh_d = din("xh", [128, DSUB, S], BF16)
    xl_d = din("xl", [128, DSUB, S], BF16)
    wqh_d = din("wqh", [128, DSUB, CPC], BF16)
    wql_d = din("wql", [128, DSUB, CPC], BF16)
    wkh_d = din("wkh", [128, DSUB, CPC], BF16)
    wkl_d = din("wkl", [128, DSUB, CPC], BF16)
    wv_d = din("wvT", [128, DSUB, CPC], BF16)
    wo_d = din("woT", [128, NPAIR, D], BF16)
    cos_d = din("cos_t", [128, S], F32)
    sin_d = din("sin_t", [128, S], F32)
    mask_d = din("mask", [128, 128], F32)
    id_d = din("ident", [128, 128], F32)
    out_d = nc.dram_tensor("out", [S, D], F32, kind="ExternalOutput").ap()

    with tile.TileContext(nc) as tc, ExitStack() as ctx:
        # ---------- persistent SBUF ----------
        pers = ctx.enter_context(tc.tile_pool(name="pers", bufs=1))

        def load(pool, dr, name):
            t = pool.tile(list(dr.shape), dr.dtype, tag=f"L{name}")
            nc.sync.dma_start(t[:], dr[:])
            return t

        wo = load(pers, wo_d, "wo")
        mask = load(pers, mask_d, "mask")
        ident = load(pers, id_d, "id")

        # band layouts (all matmul operands at base partition 0):
        # q_hl: band0 = q_hi, band1 = q_lo; k_hh: k_hi in both bands;
        # k_l: k_lo on partitions 0:64
        q_hl = pers.tile([128, NHC, S], BF16, tag="q_hl")
        k_hh = pers.tile([128, NHC, S], BF16, tag="k_hh")
        k_l = pers.tile([64, NHC, S], BF16, tag="k_l")
        v_sb = pers.tile([128, NQT, CPC], BF16, tag="v_sb")
        # unnormalized avT in O-proj lhsT layout (normalized in place later)
        av_all = pers.tile([128, NPAIR, S], BF16, tag="av_all")
        den_acc = pers.tile([128, NHC, NQT], F32, tag="den_acc")
        rec_acc = pers.tile([128, NHC, NQT], F32, tag="rec_acc")

        with tc.tile_pool(name="bload", bufs=1) as bl, \
             tc.tile_pool(name="bx", bufs=2) as bx, \
             tc.tile_pool(name="projwork", bufs=2) as pwk, \
             tc.tile_pool(name="projpsum", bufs=1, space="PSUM") as pps, \
             tc.tile_pool(name="scps", bufs=3, space="PSUM") as scps, \
             tc.tile_pool(name="avps", bufs=1, space="PSUM") as avps, \
             tc.tile_pool(name="atwork", bufs=3) as awk, \
             tc.tile_pool(name="stats", bufs=9) as stp:

            wqh = load(bl, wqh_d, "wqh")
            wql = load(bl, wql_d, "wql")
            wkh = load(bl, wkh_d, "wkh")
            wkl = load(bl, wkl_d, "wkl")
            wv = load(bl, wv_d, "wv")
            cos_t = load(bl, cos_d, "cos")
            sin_t = load(bl, sin_d, "sin")

            # ---- V projection first (dense matmuls warm the PE) ----
            for sc_i in range(S // 512):
                ssl = bass.ts(sc_i, 512)
                xv = bx.tile([128, 2, DSUB, 512], BF16, tag="xc")
                nc.sync.dma_start(xv[:, 0], xh_d[:, :, ssl])
                for st4 in range(4):
                    st = sc_i * 4 + st4
                    psv = pps.tile([128, 512], F32, tag="pp")
                    for t in range(DSUB):
                        nc.tensor.matmul(
                            psv[:, 0:CPC],
                            xv[:, 0, t, bass.ts(st4, 128)], wv[:, t, :],
                            start=(t == 0), stop=(t == DSUB - 1),
                        )
                    nc.scalar.copy(out=v_sb[:, st, :], in_=psv[:, 0:CPC])

            def proj_pair(p):
                for sc_i in range(S // 512):
                    ssl = bass.ts(sc_i, 512)
                    xc = bx.tile([128, 2, DSUB, 512], BF16, tag="xc")
                    nc.sync.dma_start(xc[:, 0], xh_d[:, :, ssl])
                    nc.sync.dma_start(xc[:, 1], xl_d[:, :, ssl])
                    # rope (2 heads stacked on partitions); q then k
                    # through one single-bank psum tile
                    for qk, (w_hi, w_lo) in enumerate(
                        ((wqh, wql), (wkh, wkl))
                    ):
                        pq = pps.tile([128, 512], F32, tag="pp")
                        n = 0
                        for t in range(DSUB):
                            for lh, xi in ((w_hi, 0), (w_hi, 1), (w_lo, 0)):
                                nc.tensor.matmul(
                                    pq[:, :],
                                    lh[:, t, bass.ts(p, 128)],
                                    xc[:, xi, t, :],
                                    start=(n == 0), stop=(n == 3 * DSUB - 1),
                                )
                                n += 1
                        f32c = pwk.tile([128, 512], F32, tag="f32c")
                        nc.scalar.copy(out=f32c[:], in_=pq[:, :])
                        swp = pwk.tile([128, 512], F32, tag="swp")
                        for a in range(2):
                            nc.sync.dma_start(
                                swp[64 * a:64 * a + 32, :],
                                f32c[64 * a + 32:64 * a + 64, :],
                            )
                            nc.sync.dma_start(
                                swp[64 * a + 32:64 * a + 64, :],
                                f32c[64 * a:64 * a + 32, :],
                            )
                        m1 = pwk.tile([128, 512], F32, tag="m1")
                        nc.gpsimd.tensor_mul(m1[:], f32c[:], cos_t[:, ssl])
                        rot = pwk.tile([128, 512], F32, tag="rot")
                        nc.vector.tensor_mul(rot[:], swp[:], sin_t[:, ssl])
                        nc.vector.tensor_add(rot[:], rot[:], m1[:])
                        for sub in range(2):
                            hh = 2 * p + sub
                            band = rot[64 * sub:64 * sub + 64, :]
                            if sub == 0:
                                b0 = band
                            else:
                                b0t = pwk.tile([64, 512], F32, tag="m1")
                                nc.vector.tensor_copy(b0t[:], band)
                                b0 = b0t[:]
                            if qk == 0:
                                nc.scalar.copy(
                                    out=q_hl[0:64, hh, ssl], in_=b0)
                                nc.vector.tensor_tensor(
                                    q_hl[64:128, hh, ssl], b0,
                                    q_hl[0:64, hh, ssl],
                                    mybir.AluOpType.subtract,
                                )
                            else:
                                nc.scalar.copy(
                                    out=k_hh[0:64, hh, ssl], in_=b0)
                                nc.scalar.copy(
                                    out=k_hh[64:128, hh, ssl], in_=b0)
                                nc.vector.tensor_tensor(
                                    k_l[0:64, hh, ssl], b0,
                                    k_hh[0:64, hh, ssl],
                                    mybir.AluOpType.subtract,
                                )

            def attn_head(hh, qt):
                p = hh // 2
                nk = (qt + 1) * 128
                qsl = bass.ts(qt, 128)
                chunks = []
                k0 = 0
                while k0 < nk:
                    chunks.append((k0, min(CHUNK, nk - k0)))
                    k0 += CHUNK
                avts = []
                nm8s = []
                accs = []
                for ci, (k0, nkc) in enumerate(chunks):
                    sc = scps.tile([128, CHUNK], F32, tag="sc")
                    for n0 in range(0, nkc, 512):
                        nn = min(512, nkc - n0)
                        ksl = bass.ds(k0 + n0, nn)
                        nc.tensor.matmul(
                            sc[:, bass.ds(n0, nn)],
                            q_hl[:, hh, qsl], k_hh[:, hh, ksl],
                            start=True, stop=False,
                        )
                        nc.tensor.matmul(
                            sc[:, bass.ds(n0, nn)],
                            q_hl[0:64, hh, qsl], k_l[:, hh, ksl],
                            start=False, stop=True,
                        )
                    if ci == len(chunks) - 1:
                        doff = nk - 128 - k0
                        nc.vector.tensor_add(
                            sc[:, bass.ds(doff, 128)],
                            sc[:, bass.ds(doff, 128)], mask[:],
                        )
                    mx = stp.tile([128, 1], F32, tag="mx")
                    nc.vector.tensor_reduce(
                        mx[:], sc[:, 0:nkc], mybir.AxisListType.X,
                        mybir.AluOpType.max,
                    )
                    nm8 = stp.tile([128, 1], F32, tag="nm8")
                    nc.vector.tensor_scalar_mul(nm8[:], mx[:], -0.125)
                    if ci > 0:
                        nmf = stp.tile([128, 1], F32, tag="nmf")
                        nc.vector.tensor_tensor(
                            nmf[:], nm8[:], nm8s[0], mybir.AluOpType.min
                        )
                        nm8 = nmf
                    nm8s.append(nm8)

                    psb = awk.tile([128, CHUNK], BF16, tag="psb")
                    if len(chunks) == 1:
                        acc = den_acc[:, hh, qt:qt + 1]
                    else:
                        acc_t = stp.tile([128, 1], F32, tag=f"acc{ci}")
                        acc = acc_t[:]
                    accs.append(acc)
                    nc.scalar.activation(
                        psb[:, 0:nkc], sc[:, 0:nkc],
                        mybir.ActivationFunctionType.Exp,
                        bias=nm8[:, 0:1], scale=0.125,
                        accum_out=acc,
                    )
                    pts = awk.tile([128, CHUNK // 128, 128], BF16, tag="pts")
                    nblk = nkc // 128
                    nc.sync.dma_start_transpose(pts[:, 0:nblk, :],
                                                psb[:, 0:nkc])
                    avt = avps.tile([65, 128], F32, tag="avt")
                    for j in range(nblk):
                        kt = (k0 + j * 128) // 128
                        nc.tensor.matmul(
                            avt[0:64, :],
                            v_sb[:, kt, bass.ds(64 * hh, 64)],
                            pts[:, j, :],
                            start=(j == 0), stop=(j == nblk - 1),
                            skip_group_check=True,
                        )
                    avts.append(avt)

                p_, h_ = hh // 2, hh % 2
                dst = av_all[64 * h_:64 * h_ + 64, p_, qsl]
                if len(avts) == 1:
                    nc.scalar.copy(out=dst, in_=avts[0][0:64, :])
                else:
                    # values: dst = avt1 * alpha1(free-dim) + avt2
                    delta = stp.tile([128, 128], BF16, tag="delta")
                    nc.vector.tensor_tensor(
                        delta[:], nm8s[1][:].to_broadcast((128, 128)),
                        nm8s[0][:].to_broadcast((128, 128)),
                        mybir.AluOpType.subtract,
                    )
                    dT = stp.tile([128, 128], BF16, tag="dT")
                    nc.sync.dma_start_transpose(dT[:], delta[:])
                    alph = stp.tile([1, 128], F32, tag="alph")
                    nc.scalar.activation(
                        alph[:], dT[0:1, :],
                        mybir.ActivationFunctionType.Exp,
                    )
                    alphb = awk.tile([64, 128], F32, tag="alphb")
                    nc.gpsimd.partition_broadcast(alphb[:], alph[0:1, :])
                    tmp = awk.tile([64, 128], F32, tag="mrg")
                    nc.vector.tensor_mul(tmp[:], avts[0][0:64, :], alphb[:])
                    nc.vector.tensor_add(dst, tmp[:], avts[1][0:64, :])
                    # denominator: den = acc2 + exp(delta)*acc1 (q on parts)
                    alq = stp.tile([128, 1], F32, tag="alq")
                    nc.scalar.activation(
                        alq[:], delta[:, 0:1],
                        mybir.ActivationFunctionType.Exp,
                    )
                    t2 = stp.tile([128, 1], F32, tag="t2")
                    nc.vector.tensor_mul(t2[:], accs[0], alq[:])
                    nc.vector.tensor_add(
                        den_acc[:, hh, qt:qt + 1], t2[:], accs[1])

            # ---- pair-pipelined main loop ----
            for step in range(NPAIR + 1):
                if step < NPAIR:
                    proj_pair(step)
                if step > 0 and debug_stop != "B":
                    for sub in range(2):
                        hh = 2 * (step - 1) + sub
                        for qt in range(NQT):
                            attn_head(hh, qt)

        # ---------- normalize + output projection ----------
        with tc.tile_pool(name="normw", bufs=2) as nwk, \
             tc.tile_pool(name="ops", bufs=2, space="PSUM") as ops, \
             tc.tile_pool(name="owork", bufs=3) as owk:
            if debug_stop != "B":
                nc.vector.reciprocal(
                    rec_acc[:].rearrange("p a b -> p (a b)"),
                    den_acc[:].rearrange("p a b -> p (a b)"),
                )
                for hh in range(NHC):
                    rT = ops.tile([NQT, 128], F32, tag="rT")
                    nc.tensor.transpose(rT[:], rec_acc[:, hh, :], ident[:])
                    rTs = nwk.tile([NQT, 128], F32, tag="rTs")
                    nc.scalar.copy(out=rTs[:], in_=rT[:])
                    p_, h_ = hh // 2, hh % 2
                    for half in range(2):
                        hq = NQT // 2
                        rech = nwk.tile([1, S // 2], F32, tag="rech")
                        nc.sync.dma_start(
                            rech[0:1, :],
                            rTs[half * hq:(half + 1) * hq, :])
                        recb = nwk.tile([128, S // 2], F32, tag="recb")
                        nc.gpsimd.partition_broadcast(
                            recb[:], rech[0:1, :])
                        sl = av_all[64 * h_:64 * h_ + 64, p_,
                                    bass.ts(half, S // 2)]
                        nc.vector.tensor_mul(
                            sl, sl, recb[64 * h_:64 * h_ + 64, :])

            for st in range(NQT):
                po = ops.tile([128, 2, 512], F32, tag="po")
                for half in range(2):
                    for p in range(NPAIR):
                        nc.tensor.matmul(
                            po[:, half, 0:384],
                            av_all[:, p, bass.ts(st, 128)],
                            wo[:, p, bass.ts(half, 384)],
                            start=(p == 0), stop=(p == NPAIR - 1),
                        )
                osb = owk.tile([128, D], F32, tag="osb")
                nc.scalar.copy(out=osb[:, 0:384], in_=po[:, 0, 0:384])
                nc.scalar.copy(out=osb[:, 384:768], in_=po[:, 1, 0:384])
                nc.sync.dma_start(out_d[bass.ts(st, 128), :], osb[:])

    nc.compile()
    return nc


def _rope_perm():
    p = np.zeros(DK, dtype=np.int64)
    for i in range(DK // 2):
        p[i] = 2 * i
        p[i + 32] = 2 * i + 1
    return p


def _split(a):
    hi = a.astype(bf16)
    lo = (a.astype(np.float32) - hi.astype(np.float32)).astype(bf16)
    return hi, lo


def _tile_din(a):
    # [768, F] -> [128, 6, F]
    return np.ascontiguousarray(a.reshape(DSUB, 128, -1).transpose(1, 0, 2))


def make_inputs(x, wq, wk, wv, wo, S):
    """Host-side prep: returns list of 8 in_maps (core = 2*b + g)."""
    perm = _rope_perm()
    pos = np.arange(S, dtype=np.float64)
    inv = 10000.0 ** (-2.0 * np.arange(DK // 2, dtype=np.float64) / DK)
    ang = pos[:, None] * inv[None, :]
    cosv = np.cos(ang).astype(np.float32).T  # [32, S]
    sinv = np.sin(ang).astype(np.float32).T
    cos_t = np.tile(cosv, (4, 1)).astype(np.float32)            # [128, S]
    sin_t = np.tile(
        np.concatenate([-sinv, sinv], axis=0), (2, 1)
    ).astype(np.float32)                                        # [128, S]
    mask = np.triu(np.full((128, 128), -1e9, np.float32), 1)
    ident = np.eye(128, dtype=np.float32)

    maps = []
    for b in range(B):
        xT = np.ascontiguousarray(x[b].T.astype(np.float32))  # [768, S]
        xh, xl = _split(xT)
        xh_t, xl_t = _tile_din(xh), _tile_din(xl)
        for g in range(2):
            hs = slice(g * CPC, (g + 1) * CPC)
            wqc = wq[hs].astype(np.float32).copy()
            wkc = wk[hs].astype(np.float32).copy()
            for arr in (wqc, wkc):
                for i in range(NHC):
                    blk = arr[i * DK:(i + 1) * DK].copy()
                    arr[i * DK:(i + 1) * DK] = blk[perm]
            wqh, wql = _split(wqc.T)  # [768, 384]
            wkh, wkl = _split(wkc.T)
            wvT = wv[hs].astype(np.float32).T.astype(bf16)
            woT = wo[:, hs].astype(np.float32).T.astype(bf16)  # [384, 768]
            maps.append({
                "xh": xh_t, "xl": xl_t,
                "wqh": _tile_din(wqh), "wql": _tile_din(wql),
                "wkh": _tile_din(wkh), "wkl": _tile_din(wkl),
                "wvT": _tile_din(wvT),
                "woT": np.ascontiguousarray(
                    woT.reshape(NPAIR, 128, D).transpose(1, 0, 2)),
                "cos_t": cos_t, "sin_t": sin_t, "mask": mask,
                "ident": ident,
            })
    return maps


_PROG = {}


def _prog(S, CHUNK):
    key = (S, CHUNK)
    if key not in _PROG:
        _PROG[key] = _build(S, CHUNK)
    return _PROG[key]


def kernel(x, wq, wk, wv, wo, S=2048, CHUNK=1024, trace=False):
    x = np.asarray(x, np.float32)
    nc = _prog(S, CHUNK)
    maps = make_inputs(x, np.asarray(wq), np.asarray(wk), np.asarray(wv),
                       np.asarray(wo), S)
    res = run_bass_kernel_spmd(nc, maps, list(range(8)), trace=trace)
    outs = []
    for b in range(B):
        outs.append(res.results[2 * b]["out"] + res.results[2 * b + 1]["out"])
    out = np.stack(outs)
    if trace:
        kernel.last_exec_time_ns = res.exec_time_ns
        kernel.last_results = res
    return out

